# revision 2
# baseline (speedup 1.0000x reference)
"""Trainium2 Bass kernel for nn_HermesMessageLayer — v2 (PE-reinjection).

Math: out[e,i,n] = sum_{b,f,r,j,m} inp[e,j,m] * precomp[e,f,r]
                                   * kernel[b,f,n,m] * weight[b,r,i,j] + bias[i]

Per 128-edge tile (data-parallel over E across 8 cores):
  [PE ] t[e, (ni,fr)] = inpT.T @ KW            (2 matmuls, 2 PSUM banks, f32)
  [Act] tsb = bf16(t)                          (one strided copy PSUM->SBUF)
  [DVE] u = tsb * pc-broadcast                 (one TT mult, 2x_1p: pc is
        stride-0-broadcast along ni; fr is the packed innermost axis)
  [DVE] v[e, (ni,5)] = u[...,0:5] + u[...,5:10]  (one TT add, pair-tree)
  [PE ] out_psum[e, ni] = sum_k I.T @ v_k + I.T @ bias_bc
        (5 strided-rhs accumulating identity matmuls + 1 bias matmul ->
         the fr-reduction happens in PSUM, evacuation is 96 not 960 elems)
  [Pool] acc = bf16(out_psum)                  (gpsimd copy, batched)
  [DMA] store acc per group; host un-permutes and upcasts.
"""

import sys

import numpy as np

sys.path.insert(0, "/opt/trn_rl_repo")

import ml_dtypes

import concourse.bass as bass
import concourse.bacc as bacc
import concourse.tile as tile
from concourse import mybir
from concourse.bass_utils import run_bass_kernel_spmd

# Problem dims
E, J, I = 300000, 32, 32
M, N = 3, 3
B, F, R = 6, 5, 2
JM = J * M          # 96
NI = I * N          # 96  (ni = i*3 + n)
FR = F * R          # 10
TCOLS = FR * NI     # 960
HB = NI // 2        # 48 ni per PSUM bank
BCOLS = HB * FR     # 480 columns per bank

NCORES = 8
E_CORE = E // NCORES            # 37500
G = 16                          # tiles per group
TILE_E = 128
GROUP_E = G * TILE_E            # 2048
NG = -(-E_CORE // GROUP_E)      # 19
E_PAD = NG * GROUP_E            # 38912
N_TILES = -(-E_CORE // TILE_E)  # 293 (last group ragged: 5 tiles)
LAST_G = N_TILES - (NG - 1) * G  # 5

BF16 = mybir.dt.bfloat16
F32 = mybir.dt.float32
_mult = mybir.AluOpType.mult
_add = mybir.AluOpType.add

POOL_NI = 36  # DVE handles ni 0..POOL_NI in the pair-tree, Pool the rest
ACT_B0 = 0    # cols of bank0 that Act also evacuates (rest go PSUM-direct)


LOOKAHEAD = 2  # stage-1 matmuls run this many tiles ahead of the back half


def build_program(ng: int = NG):
    nc = bacc.Bacc("TRN2", target_bir_lowering=False, debug=False)

    e_pad = ng * GROUP_E
    inp_t = nc.dram_tensor("inp_aug", [e_pad, 128], BF16, kind="ExternalInput").ap()
    pc_t = nc.dram_tensor("pc", [ng, 128, G, FR], BF16, kind="ExternalInput").ap()
    kw_t = nc.dram_tensor("kw", [JM, TCOLS], BF16, kind="ExternalInput").ap()
    ident_t = nc.dram_tensor("ident", [128, 128], BF16, kind="ExternalInput").ap()
    out_t = nc.dram_tensor("out", [ng, 128, G, NI], BF16, kind="ExternalOutput").ap()

    n_tiles = N_TILES if ng == NG else ng * G

    def tiles_in(g):
        return LAST_G if (ng == NG and g == ng - 1) else G

    with tile.TileContext(nc) as tc:
        with (
            tc.tile_pool(name="const", bufs=1) as const_pool,
            tc.tile_pool(name="inpT", bufs=2) as inpT_pool,
            tc.tile_pool(name="pcp", bufs=2) as pc_pool,
            tc.tile_pool(name="tsb", bufs=3) as tsb_pool,
            tc.tile_pool(name="up", bufs=3) as u_pool,
            tc.tile_pool(name="vp", bufs=3) as v_pool,
            tc.tile_pool(name="accp", bufs=2) as acc_pool,
            tc.tile_pool(name="psum", bufs=3, space="PSUM") as psum_pool,
            tc.tile_pool(name="psout", bufs=2, space="PSUM") as psout_pool,
        ):
            kw_sb = const_pool.tile([JM, TCOLS], BF16)
            ident_sb = const_pool.tile([128, 128], BF16)
            nc.sync.dma_start(kw_sb[:], kw_t[:])
            nc.sync.dma_start(ident_sb[:], ident_t[:])

            inpT_by_g = {}
            pc_by_g = {}
            acc_by_g = {}
            ps_by_t = {}
            ops4_by_g = {}

            for t_idx in range(n_tiles + LOOKAHEAD):
                if t_idx < n_tiles:
                    g = min(t_idx // G, ng - 1)
                    gi = t_idx - g * G
                    if gi == 0:
                        tg = tiles_in(g)
                        inpT = inpT_pool.tile([128, GROUP_E], BF16)
                        nc.sync.dma_start(
                            inpT[:, 0 : tg * TILE_E],
                            inp_t[g * GROUP_E : g * GROUP_E + tg * TILE_E, :],
                            transpose=True,
                        )
                        inpT_by_g[g] = inpT
                        pc = pc_pool.tile([128, G, FR], BF16)
                        nc.sync.dma_start(pc[:, 0:tg], pc_t[g][:, 0:tg])
                        pc_by_g[g] = pc
                        acc = acc_pool.tile([128, G, NI], BF16, name=f"acc{g % 2}")
                        acc_by_g[g] = acc
                        if g >= 2:
                            del inpT_by_g[g - 2], pc_by_g[g - 2], acc_by_g[g - 2]

                    ps = psum_pool.tile([128, 1024], F32)
                    ps_by_t[t_idx] = ps
                    lhsT = inpT_by_g[g][0:JM, gi * TILE_E : (gi + 1) * TILE_E]
                    nc.tensor.matmul(
                        ps[:, 0:BCOLS], lhsT, kw_sb[:, 0:BCOLS], start=True, stop=True
                    )
                    nc.tensor.matmul(
                        ps[:, 512 : 512 + BCOLS],
                        lhsT,
                        kw_sb[:, BCOLS:TCOLS],
                        start=True,
                        stop=True,
                    )

                if t_idx >= LOOKAHEAD:
                    k_idx = t_idx - LOOKAHEAD
                    kg = min(k_idx // G, ng - 1)
                    kgi = k_idx - kg * G
                    k_last = tiles_in(kg) - 1
                    ps = ps_by_t.pop(k_idx)

                    # Act: evacuate bank0-tail then bank1 into contiguous tsb
                    nb0 = ACT_B0 // FR  # ni's of bank0 Act covers
                    d_cols = BCOLS - ACT_B0  # cols DVE reads PSUM-direct
                    tsb = tsb_pool.tile([128, BCOLS + ACT_B0], BF16)
                    if ACT_B0:
                        nc.scalar.copy(tsb[:, 0:ACT_B0], ps[:, d_cols:BCOLS])
                    nc.scalar.copy(
                        tsb[:, ACT_B0:], ps[:, 512 : 512 + BCOLS]
                    )

                    pcb_d = (
                        pc_by_g[kg][:, kgi]
                        .unsqueeze(1)
                        .broadcast_to([128, HB - nb0, FR])
                    )
                    pcb_s = (
                        pc_by_g[kg][:, kgi]
                        .unsqueeze(1)
                        .broadcast_to([128, HB + nb0, FR])
                    )
                    # DVE: u head PSUM-direct; the rest from contiguous tsb
                    u = u_pool.tile([128, TCOLS], BF16)
                    u3 = u[:].rearrange("p (n f) -> p n f", f=FR)
                    ps0 = ps[:, 0:d_cols].rearrange("p (n f) -> p n f", f=FR)
                    nc.vector.tensor_tensor(
                        u3[:, 0 : HB - nb0], ps0, pcb_d, op=_mult
                    )
                    t1 = tsb[:].rearrange("p (n f) -> p n f", f=FR)
                    nc.vector.tensor_tensor(
                        u3[:, HB - nb0 : NI], t1, pcb_s, op=_mult
                    )

                    # pair-tree 10 -> 5 (DVE ni 0..POOL_NI, Pool the rest)
                    v = v_pool.tile([128, NI, 5], BF16)
                    nc.vector.tensor_tensor(
                        v[:, 0:POOL_NI, :],
                        u3[:, 0:POOL_NI, 0:5],
                        u3[:, 0:POOL_NI, 5:10],
                        op=_add,
                    )
                    if POOL_NI < NI:
                        nc.gpsimd.tensor_tensor(
                            v[:, POOL_NI:NI, :],
                            u3[:, POOL_NI:NI, 0:5],
                            u3[:, POOL_NI:NI, 5:10],
                            op=_add,
                        )

                    # PE: single 480-col reinject, fr-sum via PSUM accumulate
                    q = kgi % 4
                    if q == 0:
                        ops4 = psout_pool.tile([128, 4 * NI], F32, name="ops4")
                        ops4_by_g[kg] = ops4
                    ops4 = ops4_by_g[kg]
                    dst = ops4[:, q * NI : (q + 1) * NI]
                    rhs = v[:].rearrange("p n k -> p k n")
                    out_v = dst.unsqueeze(1).broadcast_to([128, 5, NI])
                    nc.tensor.matmul(out_v, ident_sb[:], rhs, start=True, stop=True)

                    # Act: evacuate out, batched over up to 4 tiles
                    if q == 3 or kgi == k_last:
                        nb = q + 1
                        a4 = (
                            acc_by_g[kg][:, kgi - q : kgi + 1]
                            .rearrange("p g n -> p (g n)")
                        )
                        nc.scalar.copy(a4, ops4[:, 0 : nb * NI])

                    if kgi == k_last:
                        tg = tiles_in(kg)
                        nc.sync.dma_start(
                            out_t[kg][:, 0:tg], acc_by_g[kg][:, 0:tg]
                        )

    nc.compile()
    return nc


def _pack_core(inp_c, precomp_c, ng: int = NG):
    e_pad = ng * GROUP_E
    e_c = inp_c.shape[0]
    inp_aug = np.zeros([e_pad, 128], dtype=ml_dtypes.bfloat16)
    inp_aug[:e_c, :JM] = inp_c.reshape(e_c, JM).astype(ml_dtypes.bfloat16)

    pc_pad = np.zeros([e_pad, FR], dtype=ml_dtypes.bfloat16)
    pc_pad[:e_c] = precomp_c.reshape(e_c, FR).astype(ml_dtypes.bfloat16)
    pc_perm = np.ascontiguousarray(
        pc_pad.reshape(ng, G, TILE_E, FR).transpose(0, 2, 1, 3)
    )
    return inp_aug, pc_perm


def _pack_shared(kernel, weight):
    # KW[(j,m), (ni, fr)] ni-major, fr innermost; bank-split on ni (48+48)
    kw = np.einsum(
        "bfnm,brij->jminfr",
        kernel.astype(np.float64),
        weight.astype(np.float64),
    ).reshape(JM, NI, FR)
    # column layout: bank b cols = ni' * FR + fr, ni = b*48 + ni'
    kw_cols = kw.reshape(JM, TCOLS)
    kw_b = kw_cols.astype(ml_dtypes.bfloat16)
    ident = np.eye(128, dtype=np.float32).astype(ml_dtypes.bfloat16)
    return kw_b, ident


_PROGRAM_CACHE = {}


def _get_program(ng: int = NG):
    if ng not in _PROGRAM_CACHE:
        _PROGRAM_CACHE[ng] = build_program(ng)
    return _PROGRAM_CACHE[ng]


def _make_in_maps(inp, precomp, kernel_np, weight, bias):
    kw_b, ident = _pack_shared(kernel_np, weight)
    in_maps = []
    for c in range(NCORES):
        sl = slice(c * E_CORE, (c + 1) * E_CORE)
        inp_aug, pc_perm = _pack_core(inp[sl], precomp[sl])
        in_maps.append(
            {
                "inp_aug": inp_aug,
                "pc": pc_perm,
                "kw": kw_b,
                "ident": ident,
            }
        )
    return in_maps


def kernel(inp, precomp, kernel, weight, bias):
    inp = np.asarray(inp)
    precomp = np.asarray(precomp)
    kernel_np = np.asarray(kernel)
    weight = np.asarray(weight)
    bias = np.asarray(bias)

    in_maps = _make_in_maps(inp, precomp, kernel_np, weight, bias)
    nc = _get_program()
    res = run_bass_kernel_spmd(nc, in_maps, list(range(NCORES)))

    out = np.empty([E, I, N], dtype=np.float32)
    for c in range(NCORES):
        o = np.asarray(res.results[c]["out"]).astype(np.float32)  # [NG,128,G,NI]
        o = o.transpose(0, 2, 1, 3).reshape(NG * GROUP_E, NI)[:E_CORE]
        out[c * E_CORE : (c + 1) * E_CORE] = o.reshape(E_CORE, I, N)
    out += bias.astype(np.float32)[None, :, None]
    return out
